# revision 12
# baseline (speedup 1.0000x reference)
"""GNN NodeBlock kernel for 8 TRN2 NeuronCores.

Math: out = (segment_mean(edge_attrs @ Wp + bp, dst)) @ Wu + bu, with bp=bu=0.
Projection is linear, so segment_sum(X @ Wp) == segment_sum(X) @ Wp and the
two MLPs fuse into one 64x64 weight Wf = Wp @ Wu applied to the aggregate.

Sharding: node-parallel. The host sorts edges by destination (a pure layout /
sharding permutation) and gives each core a contiguous node range plus its
edges, grouped into tiles of <=128 consecutive nodes with a fixed edge-chunk
budget. No collectives are needed.

Device (SPMD, identical program on 8 cores):
  per 128-edge chunk:  onehot[e, r] = (iota[r] == local_dst[e])   (DVE, bf16)
                       PSUM[0:65, 0:128] += Xaug^T @ onehot       (TensorE)
                       where Xaug = [x | 1] so row 64 accumulates counts
  per tile:            S = PSUM -> SBUF (bf16), then one matmul
                       S^T @ [Wf | e_cnt] -> [128 nodes, 64 feat | counts]
                       out = mlp[:, :64] * (1 / max(counts, 1))   per-partition
Host then scatters per-tile rows back to global node ids (pure permutation).
"""

import sys

sys.path.insert(0, "/opt/trn_rl_repo")

import numpy as np

P = 128
D = 64
NCORES = 8
CAP_CHUNKS_MIN = 16  # edge-chunk budget per node tile (16*128 = 2048 slots)


def _plan_tiles(counts, cum, s, cap):
    """Greedy tiles per core: consecutive nodes, <=128 nodes, <=cap edges."""
    core_tiles = []
    for k in range(NCORES):
        a = int(s[k])
        hi = int(s[k + 1])
        tiles = []
        while a < hi:
            b_lim = min(a + P, hi)
            b = int(np.searchsorted(cum, cum[a] + cap, side="right")) - 1
            b = max(a + 1, min(b, b_lim))
            tiles.append((a, b))
            a = b
        core_tiles.append(tiles)
    return core_tiles


def _build_program(nt, cap_chunks):
    import concourse.bacc as bacc
    from concourse import mybir
    from concourse.masks import make_identity
    from concourse.tile import TileContext

    BF = mybir.dt.bfloat16
    F32 = mybir.dt.float32
    nchunk = nt * cap_chunks
    DA = D + 1  # attrs + ones column

    nc = bacc.Bacc(None, target_bir_lowering=False)
    x_d = nc.declare_dram_parameter("x", [P, nchunk * DA], BF, isOutput=False)
    ldst_d = nc.declare_dram_parameter("ldst", [P, nchunk], F32, isOutput=False)
    wp_d = nc.declare_dram_parameter("wp", [D, D], F32, isOutput=False)
    wu_d = nc.declare_dram_parameter("wu", [D, D], F32, isOutput=False)
    out_d = nc.declare_dram_parameter("out", [nt * P, D], F32, isOutput=True)

    with TileContext(nc) as tc:
        with (
            tc.tile_pool(name="const", bufs=1) as cp,
            tc.tile_pool(name="xin", bufs=4) as xp,
            tc.tile_pool(name="oh", bufs=64) as ohp,
            tc.tile_pool(name="flush", bufs=4) as fp,
            tc.tile_pool(name="small", bufs=8) as sp,
            tc.tile_pool(name="d2", bufs=4) as dp,
            tc.tile_pool(name="res", bufs=2) as rp,
            tc.tile_pool(name="psacc", bufs=3, space="PSUM") as psa,
            tc.tile_pool(name="psmlp", bufs=2, space="PSUM") as psb,
            tc.tile_pool(name="psw", bufs=1, space="PSUM") as psw,
        ):
            # --- constants ---
            ident = cp.tile([D, D], F32)
            make_identity(nc, ident[:])
            iota_i = cp.tile([P, P], mybir.dt.int32)
            nc.gpsimd.iota(iota_i[:], pattern=[[1, P]], base=0, channel_multiplier=0)
            iota_bf = cp.tile([P, P], BF)
            nc.vector.tensor_copy(iota_bf[:], iota_i[:])
            ldst_sb = cp.tile([P, nchunk], F32)
            nc.sync.dma_start(out=ldst_sb[:], in_=ldst_d[:])
            ldst_neg = cp.tile([P, nchunk], F32)
            nc.vector.tensor_scalar_mul(ldst_neg[:], ldst_sb[:], -1.0)

            # --- fused weight Wf = Wp @ Wu, extended with a count column ---
            wp_sb = cp.tile([D, D], F32)
            nc.sync.dma_start(out=wp_sb[:], in_=wp_d[:])
            wu_sb = cp.tile([D, D], F32)
            nc.sync.dma_start(out=wu_sb[:], in_=wu_d[:])
            wpt_ps = psw.tile([D, D], F32)
            nc.tensor.transpose(out=wpt_ps[:], in_=wp_sb[:], identity=ident[:])
            wpt_bf = cp.tile([D, D], BF)
            nc.vector.tensor_copy(wpt_bf[:], wpt_ps[:])
            wu_bf = cp.tile([D, D], BF)
            nc.vector.tensor_copy(wu_bf[:], wu_sb[:])
            wf_ps = psw.tile([D, D], F32)
            nc.tensor.matmul(wf_ps[:], lhsT=wpt_bf[:], rhs=wu_bf[:], start=True, stop=True)
            wf_ext = cp.tile([DA, DA], BF)
            nc.gpsimd.memset(wf_ext[:], 0.0)
            nc.vector.tensor_copy(wf_ext[0:D, 0:D], wf_ps[:])
            nc.gpsimd.memset(wf_ext[D : D + 1, D : D + 1], 1.0)

            # --- main loop ---
            XB = 2  # tiles per input DMA
            OB = 4  # tiles per output DMA
            xins = {}
            outbs = {}
            for t in range(nt):
                if t % XB == 0:
                    span = min(XB, nt - t)
                    xin = xp.tile([P, XB * cap_chunks * DA], BF, tag="xin", name=f"xin{t}")
                    nc.sync.dma_start(
                        out=xin[:, : span * cap_chunks * DA],
                        in_=x_d[:, t * cap_chunks * DA : (t + span) * cap_chunks * DA],
                    )
                    xins[t] = xin
                xin = xins[t - t % XB]
                xoff = (t % XB) * cap_chunks * DA
                acc = psa.tile([DA, P], F32)
                for c in range(cap_chunks):
                    gc = t * cap_chunks + c
                    oh = ohp.tile([P, P], BF)
                    if gc % 20 == 9:
                        # ACT path: onehot = relu(1 - (iota - ldst)^2)
                        d2 = dp.tile([P, P], BF)
                        nc.scalar.activation(
                            out=d2[:], in_=iota_bf[:],
                            func=mybir.ActivationFunctionType.Square,
                            bias=ldst_neg[:, gc : gc + 1], scale=1.0,
                        )
                        nc.scalar.activation(
                            out=oh[:], in_=d2[:],
                            func=mybir.ActivationFunctionType.Relu,
                            bias=1.0, scale=-1.0,
                        )
                    else:
                        oh_eng = nc.gpsimd if (c % 4 == 3) else nc.vector
                        oh_eng.tensor_scalar(
                            out=oh[:],
                            in0=iota_bf[:],
                            scalar1=ldst_sb[:, gc : gc + 1],
                            scalar2=None,
                            op0=mybir.AluOpType.is_equal,
                        )
                    nc.tensor.matmul(
                        acc[:],
                        lhsT=xin[:, xoff + c * DA : xoff + (c + 1) * DA],
                        rhs=oh[:],
                        start=(c == 0),
                        stop=(c == cap_chunks - 1),
                    )
                s_bf = fp.tile([DA, P], BF)
                nc.scalar.copy(out=s_bf[:], in_=acc[:])
                mlp = psb.tile([P, DA], F32)
                nc.tensor.matmul(mlp[:], lhsT=s_bf[:], rhs=wf_ext[:], start=True, stop=True)
                cnt_m = sp.tile([P, 1], F32)
                nc.vector.tensor_scalar_max(cnt_m[:], mlp[:, D : D + 1], 1.0)
                recip = sp.tile([P, 1], F32)
                nc.vector.reciprocal(recip[:], cnt_m[:])
                if t % OB == 0:
                    outbs[t] = rp.tile([P, OB * D], F32, tag="outb", name=f"outb{t}")
                outb = outbs[t - t % OB]
                g = t % OB
                nc.scalar.activation(
                    out=outb[:, g * D : (g + 1) * D],
                    in_=mlp[:, 0:D],
                    func=mybir.ActivationFunctionType.Copy,
                    scale=recip[:, 0:1],
                )
                if t % OB == OB - 1 or t == nt - 1:
                    t0 = t - g
                    span = g + 1
                    dst_ap = out_d[t0 * P : (t0 + span) * P, :].rearrange(
                        "(g p) f -> p g f", p=P
                    )
                    nc.sync.dma_start(out=dst_ap, in_=outb[:, : span * D].rearrange("p (g f) -> p g f", f=D))

    return nc


def _prepare(inputs):
    """Host-side shard/layout prep. Returns (in_maps, core_tiles, nt, cap_chunks, N)."""
    from concourse import mybir

    bf16 = mybir.dt.np(mybir.dt.bfloat16)

    edge_attrs = np.asarray(inputs["edge_attrs"], dtype=np.float32)
    wp = np.ascontiguousarray(np.asarray(inputs["proj_W"], dtype=np.float32))
    wu = np.ascontiguousarray(np.asarray(inputs["upd_W"], dtype=np.float32))
    dst = np.asarray(inputs["dst"]).astype(np.int64).ravel()
    N = int(np.asarray(inputs["n_nodes"]))
    E = dst.shape[0]

    perm = np.argsort(dst, kind="stable")
    sdst = dst[perm]
    sx = edge_attrs[perm].astype(bf16)

    counts = np.bincount(sdst, minlength=N)
    cum = np.concatenate([[0], np.cumsum(counts)])

    # node-aligned, roughly equal-edge core split
    s = [0]
    for k in range(1, NCORES):
        s.append(int(sdst[min((k * E) // NCORES, E - 1)]))
    s.append(N)
    s = np.maximum.accumulate(np.asarray(s, dtype=np.int64))

    cap_chunks = max(CAP_CHUNKS_MIN, int(np.ceil(counts.max() / P))) if E else CAP_CHUNKS_MIN
    cap = cap_chunks * P
    core_tiles = _plan_tiles(counts, cum, s, cap)
    nt = max(len(t) for t in core_tiles)
    nchunk = nt * cap_chunks
    DA = D + 1

    in_maps = []
    for k in range(NCORES):
        x_core = np.zeros((nchunk * P, DA), dtype=bf16)
        x_core[:, D] = 1.0
        ldst_core = np.full((nchunk * P,), 1000.0, dtype=np.float32)
        for t, (a, b) in enumerate(core_tiles[k]):
            e0, e1 = int(cum[a]), int(cum[b])
            n = e1 - e0
            base = t * cap * 1
            x_core[base : base + n, :D] = sx[e0:e1]
            ldst_core[base : base + n] = (sdst[e0:e1] - a).astype(np.float32)
        x_dev = np.ascontiguousarray(
            x_core.reshape(nchunk, P, DA).transpose(1, 0, 2).reshape(P, nchunk * DA)
        )
        ldst_dev = np.ascontiguousarray(ldst_core.reshape(nchunk, P).T)
        in_maps.append({"x": x_dev, "ldst": ldst_dev, "wp": wp, "wu": wu})

    return in_maps, core_tiles, nt, cap_chunks, N


def kernel(**inputs) -> np.ndarray:
    from concourse.bass_utils import run_bass_kernel_spmd

    in_maps, core_tiles, nt, cap_chunks, N = _prepare(inputs)
    nc = _build_program(nt, cap_chunks)
    nc.finalize()
    res = run_bass_kernel_spmd(nc, in_maps, core_ids=list(range(NCORES)))

    out_full = np.zeros((N, D), dtype=np.float32)
    for k in range(NCORES):
        o = res.results[k]["out"]
        for t, (a, b) in enumerate(core_tiles[k]):
            out_full[a:b] = o[t * P : t * P + (b - a)]
    return out_full
